# revision 2
# baseline (speedup 1.0000x reference)
"""TransformerConv MixerBlock (x + TransformerConv(x, edge_index)) on 8 trn2 NeuronCores.

v2: same tile/bin-pack strategy as baseline, but:
 - per-tile kv gathers batched into 2 dma_gather custom instructions (int16
   indices, low/high table-range split at a 128-aligned point — possible
   because per-tile edges are sorted by source slot)
 - host-side pre-transposed x (no DMA transpose), host-side (dh,h) column
   permutation so broadcast multiplies hit DVE 2x mode
 - pqe copied PSUM->SBUF fp16 on scalar engine so qk multiply runs 2x
 - outputs accumulated in SBUF, one DMA at the end; fp16 output
"""
import sys, os, types, math, heapq
sys.path.insert(0, '/opt/trn_rl_repo')
import numpy as np

P = 128
D = 128
H = 4
DH = 32
NCORES = 8
JB = 8              # chunks in the low-range gather segment
SPLIT_LO = 32768    # gather A covers kv_table[0:SPLIT_LO)
SPLIT_HI = 17408    # gather B covers kv_table[SPLIT_HI:)
GATHER_MODE = "dma_gather"   # or "indirect"

_prog_cache = {}


def _ensure_hooks():
    """Best-effort shim of antenv.axon_hooks so trace=True profiling works."""
    try:
        import antenv
        if 'antenv.axon_hooks' not in sys.modules:
            mod = types.ModuleType('antenv.axon_hooks')
            state = {'hook': None}
            mod.set_axon_ntff_profile_hook = lambda h: state.__setitem__('hook', h)
            mod.get_axon_ntff_profile_hook = lambda: state['hook']
            sys.modules['antenv.axon_hooks'] = mod
            antenv.axon_hooks = mod
            from trn_agent_boot.trn_boot import _ntff_profile_via_ctypes
            hook = _ntff_profile_via_ctypes('/opt/axon/libaxon_pjrt.so')
            if hook is not None:
                mod.set_axon_ntff_profile_hook(hook)
    except Exception:
        pass
    try:
        import concourse.bass_utils as bass_utils
        bass_utils.upload_artifacts = lambda tmpdir: tmpdir
    except Exception:
        pass


def _prep(x, edge_index, Wq, bq, Wk, bk, Wv, bv, Wskip, bskip):
    N = x.shape[0]
    E = edge_index.shape[1]
    TPC = (N + NCORES * P - 1) // (NCORES * P)
    NT = NCORES * TPC

    src = np.asarray(edge_index[0], dtype=np.int64)
    dst = np.asarray(edge_index[1], dtype=np.int64)
    deg = np.bincount(dst, minlength=N)

    # --- bin-pack nodes into NT tiles of <=P nodes, balancing degree sums ---
    order = np.argsort(-deg, kind='stable')
    heap = [(0, t) for t in range(NT)]
    heapq.heapify(heap)
    counts = np.zeros(NT, dtype=np.int64)
    tile_deg = np.zeros(NT, dtype=np.int64)
    node_slot = np.empty(N, dtype=np.int64)
    for n in order:
        while True:
            dsum, t = heapq.heappop(heap)
            if counts[t] < P:
                break
        node_slot[n] = t * P + counts[t]
        counts[t] += 1
        tile_deg[t] += deg[n]
        if counts[t] < P:
            heapq.heappush(heap, (dsum + int(deg[n]), t))
    K = max(JB + 1, int((tile_deg.max() + P - 1) // P))
    KP = K * P
    JBP = JB * P

    # --- (dh, h) column permutation: new col f' = dh*H + h  <-  old h*DH + dh
    old = np.empty(D, dtype=np.int64)
    for f in range(D):
        dh, hh = divmod(f, H)
        old[f] = hh * DH + dh

    # --- permuted node features (original column order) + transpose ---
    x_perm = np.zeros((NT * P, D), dtype=np.float16)
    x_perm[node_slot] = np.asarray(x, dtype=np.float16)
    xT = np.ascontiguousarray(x_perm.T)                   # [D, NT*P]
    xP = np.ascontiguousarray(x_perm[:, old])             # [NT*P, D] permuted cols

    # --- per-tile edge lists sorted by src slot ---
    src_slot = node_slot[src]
    dst_slot = node_slot[dst]
    et = dst_slot // P
    key = et * (1 << 32) + src_slot
    eorder = np.argsort(key, kind='stable')
    et_s = et[eorder]
    src_s = src_slot[eorder]
    dloc_s = dst_slot[eorder] - et_s * P

    ecnt = np.bincount(et, minlength=NT)
    eoff = np.zeros(NT + 1, dtype=np.int64)
    np.cumsum(ecnt, out=eoff[1:])

    idx_pad = np.zeros((NT, KP), dtype=np.int64)
    idx_pad[:, JBP:] = SPLIT_HI
    dloc_pad = np.full((NT, KP), 255, dtype=np.int64)
    for t in range(NT):
        lo_, hi_ = int(eoff[t]), int(eoff[t + 1])
        ss = src_s[lo_:hi_]
        dd = dloc_s[lo_:hi_]
        ec = hi_ - lo_
        n2 = int(np.searchsorted(ss, SPLIT_LO))
        sA = min(JBP, n2)
        if sA < ec:
            assert ss[sA] >= SPLIT_HI, f"tile {t}: low/high split infeasible"
        assert ec - sA <= KP - JBP, f"tile {t}: high segment overflow"
        idx_pad[t, :sA] = ss[:sA]
        dloc_pad[t, :sA] = dd[:sA]
        nB = ec - sA
        idx_pad[t, JBP:JBP + nB] = ss[sA:]
        dloc_pad[t, JBP:JBP + nB] = dd[sA:]
    idx_pad[:, JBP:] -= SPLIT_HI
    assert idx_pad.min() >= 0 and idx_pad.max() < SPLIT_LO

    # int16 indices wrapped in 16 partitions (pos i -> [i%16, i//16]), and the
    # two segments concatenated along columns; replicated over all 128 parts
    pr = np.arange(P) % 16
    posA = np.arange(JBP // 16)[None, :] * 16 + pr[:, None]          # [P, 64]
    posB = JBP + np.arange((KP - JBP) // 16)[None, :] * 16 + pr[:, None]
    pos = np.concatenate([posA, posB], axis=1)                        # [P, KP//16]
    i16w = idx_pad[:, pos].astype(np.int16)                           # [NT, P, KP//16]

    # int32 per-chunk layout for the indirect fallback: [NT, P, K]
    src_g = idx_pad.copy()
    src_g[:, JBP:] += SPLIT_HI
    src_g = src_g.reshape(NT, K, P).transpose(0, 2, 1).astype(np.int32).copy()

    dlv = dloc_pad.reshape(NT, K, P).transpose(0, 2, 1).astype(np.float32).copy()
    ixdl = np.concatenate(
        [i16w, dlv.view(np.int16)], axis=2)                           # [NT, P, KP//16 + 2K]

    oh = (dloc_pad.reshape(NT, 1, KP) == np.arange(P).reshape(1, P, 1))
    oh = oh.astype(np.float16)
    # transposed one-hot: ohT[t, p, c*P + r] = 1 iff dloc(edge c*P+p) == r
    dl3 = dloc_pad.reshape(NT, K, P)                                  # [t, c, p]
    ohT = (dl3.transpose(0, 2, 1)[:, :, :, None] ==
           np.arange(P).reshape(1, 1, 1, P)).astype(np.float16)       # [t, p, c, r]
    ohT = ohT.reshape(NT, P, KP)

    s = 1.0 / math.sqrt(DH)
    wkT = np.ascontiguousarray(np.asarray(Wk, np.float32)[old].T).astype(np.float16)
    wvT = np.ascontiguousarray(np.asarray(Wv, np.float32)[old].T).astype(np.float16)
    wqT = np.ascontiguousarray(np.asarray(Wq, np.float32)[old].T * s).astype(np.float16)
    wsT = np.ascontiguousarray(np.asarray(Wskip, np.float32)[old].T).astype(np.float16)
    for b in (bq, bk, bv, bskip):
        assert np.abs(np.asarray(b)).max() == 0.0, "nonzero biases not supported"
    iota = np.tile(np.arange(P, dtype=np.float16).reshape(1, P), (P, 1)).copy()

    # host-rearranged local x (permuted cols): xlp_r[p, u*D + c] = xP[t0*P + u*P + p, c]
    in_maps = []
    for c in range(NCORES):
        t0, t1 = c * TPC, (c + 1) * TPC
        xlp_r = xP[t0 * P:t1 * P].reshape(TPC, P, D).transpose(1, 0, 2).reshape(P, TPC * D).copy()
        im = {
            "xt_full": xT,
            "xt_loc": xT[:, t0 * P:t1 * P].copy(),
            "xlp": xlp_r,
            "wkvT": np.concatenate([wkT, wvT], axis=1).copy(),
            "wqsT": np.concatenate([wqT, wsT], axis=1).copy(),
            "iota": iota,
            "ixdl": ixdl[t0:t1].reshape(TPC * P, -1).copy(),
            "oh": oh[t0:t1].reshape(TPC * P, KP).copy(),
            "ohT": ohT[t0:t1].reshape(TPC * P, KP).copy(),
        }
        if GATHER_MODE != "dma_gather":
            im["src_idx"] = src_g[t0:t1].reshape(TPC * P, K).copy()
        in_maps.append(im)
    return dict(N=N, E=E, TPC=TPC, NT=NT, K=K, node_slot=node_slot, old=old,
                in_maps=in_maps)


def _build(TPC, NT, K):
    import concourse.bass as bass
    import concourse.bacc as bacc
    import concourse.mybir as mybir
    import concourse.tile as tile
    from concourse import library_config

    f16 = mybir.dt.float16
    f32 = mybir.dt.float32
    i16 = mybir.dt.int16
    i32 = mybir.dt.int32
    MUL = mybir.AluOpType.mult
    ADD = mybir.AluOpType.add
    ISEQ = mybir.AluOpType.is_equal
    EXP = mybir.ActivationFunctionType.Exp
    COPY = mybir.ActivationFunctionType.Copy

    KP = K * P
    JBP = JB * P
    NTP = NT * P
    IXC = KP // 16            # idx16 columns
    NBS = 16                  # node superblock (tiles per DMA)
    NB = 2                    # node compute block (tiles per PSUM tile)

    nc = bacc.Bacc("TRN2", target_bir_lowering=False, debug=False)
    xt_full = nc.dram_tensor("xt_full", [D, NTP], f16, kind="ExternalInput")
    xt_loc = nc.dram_tensor("xt_loc", [D, TPC * P], f16, kind="ExternalInput")
    xlp = nc.dram_tensor("xlp", [P, TPC * D], f16, kind="ExternalInput")
    wkvT = nc.dram_tensor("wkvT", [D, 256], f16, kind="ExternalInput")
    wqsT = nc.dram_tensor("wqsT", [D, 256], f16, kind="ExternalInput")
    iota = nc.dram_tensor("iota", [P, P], f16, kind="ExternalInput")
    ixdl = nc.dram_tensor("ixdl", [TPC * P, IXC + 2 * K], i16, kind="ExternalInput")
    if GATHER_MODE != "dma_gather":
        src_idx = nc.dram_tensor("src_idx", [TPC * P, K], i32, kind="ExternalInput")
    oh_in = nc.dram_tensor("oh", [TPC * P, KP], f16, kind="ExternalInput")
    ohT_in = nc.dram_tensor("ohT", [TPC * P, KP], f16, kind="ExternalInput")
    out_t = nc.dram_tensor("out", [P, TPC * D], f16, kind="ExternalOutput")

    kv_table = nc.dram_tensor("kv_table", [NTP, 256], f16)

    with tile.TileContext(nc) as tc:
        if GATHER_MODE == "dma_gather":
            nc.gpsimd.load_library(library_config.mlp)
        with (
            tc.tile_pool(name="const", bufs=1) as cp,
            tc.tile_pool(name="sbuf", bufs=4) as sb,
            tc.tile_pool(name="nsb", bufs=2) as nsb,
            tc.tile_pool(name="big", bufs=3) as bigp,
            tc.tile_pool(name="psN", bufs=2, space="PSUM") as psN,
            tc.tile_pool(name="psQ", bufs=2, space="PSUM") as psQ,
            tc.tile_pool(name="psS", bufs=2, space="PSUM") as psS,
        ):
            wkv_sb = cp.tile([D, 256], f16, tag="wkv")
            wqs_sb = cp.tile([D, 256], f16, tag="wqs")
            iota_sb = cp.tile([P, P], f16, tag="iota")
            q_loc = cp.tile([P, TPC * D], f16, tag="qloc")
            s_loc = cp.tile([P, TPC * D], f16, tag="sloc")
            out_acc = cp.tile([P, TPC * D], f16, tag="oacc")
            nc.sync.dma_start(out=wkv_sb[:], in_=wkvT[:])
            nc.sync.dma_start(out=wqs_sb[:], in_=wqsT[:])
            nc.sync.dma_start(out=iota_sb[:], in_=iota[:])

            # ---------------- local phase: q and skip ----------------
            xloc_sb = cp.tile([P, TPC * P], f16, tag="xloc")
            nc.sync.dma_start(out=xloc_sb[:], in_=xt_loc[:])
            xlp_sb = cp.tile([P, TPC * D], f16, tag="xlp")
            nc.sync.dma_start(out=xlp_sb[:], in_=xlp[:])
            u = 0
            while u < TPC:
                lb = min(NB, TPC - u)
                pq = psN.tile([P, NB, 256], f32, tag="pn")
                for b in range(lb):
                    nc.tensor.matmul(pq[:, b, :],
                                     lhsT=xloc_sb[:, (u + b) * P:(u + b + 1) * P],
                                     rhs=wqs_sb[:], start=True, stop=True)
                nc.scalar.activation(
                    out=q_loc[:, u * D:(u + lb) * D].rearrange(
                        "p (b c) -> p b c", c=D),
                    in_=pq[:, :lb, 0:128], func=COPY)
                nc.vector.tensor_tensor(
                    out=s_loc[:, u * D:(u + lb) * D].rearrange(
                        "p (b c) -> p b c", c=D),
                    in0=pq[:, :lb, 128:256],
                    in1=xlp_sb[:, u * D:(u + lb) * D].rearrange(
                        "p (b c) -> p b c", c=D),
                    op=ADD)
                u += lb

            # ---------------- node phase: full kv table ----------------
            t0 = 0
            while t0 < NT:
                nbs = min(NBS, NT - t0)
                xb = nsb.tile([P, NBS * P], f16, tag="xb")
                nc.sync.dma_start(
                    out=xb[:, :nbs * P],
                    in_=xt_full[:, t0 * P:(t0 + nbs) * P])
                kvt = nsb.tile([P, NBS, 256], f16, tag="kvt")
                for ib in range(nbs // NB):
                    pkv = psN.tile([P, NB, 256], f32, tag="pn")
                    for b in range(NB):
                        nc.tensor.matmul(
                            pkv[:, b, :],
                            lhsT=xb[:, (ib * NB + b) * P:(ib * NB + b + 1) * P],
                            rhs=wkv_sb[:], start=True, stop=True)
                    nc.scalar.activation(
                        out=kvt[:, ib * NB:(ib + 1) * NB, :], in_=pkv[:, :, :],
                        func=COPY)
                nc.sync.dma_start(
                    out=kv_table[t0 * P:(t0 + nbs) * P, :].rearrange(
                        "(b p) c -> p b c", p=P),
                    in_=kvt[:, :nbs, :])
                t0 += nbs

            # ---------------- edge phase ----------------
            groups = []
            c0 = 0
            while c0 < K:
                groups.append((c0, min(8, K - c0)))
                c0 += 8
            for u in range(TPC):
                ixdl_sb = sb.tile([P, IXC + 2 * K], i16, tag="ix")
                nc.sync.dma_start(out=ixdl_sb[:],
                                  in_=ixdl[u * P:(u + 1) * P, :])
                oh = bigp.tile([P, KP], f16, tag="oh")
                nc.sync.dma_start(out=oh[:], in_=oh_in[u * P:(u + 1) * P, :])
                ohT = bigp.tile([P, K, P], f16, tag="ohT")
                nc.sync.dma_start(
                    out=ohT[:].rearrange("p a r -> p (a r)"),
                    in_=ohT_in[u * P:(u + 1) * P, :])
                kv_g = bigp.tile([P, K, 256], f16, tag="kvg")
                if GATHER_MODE == "dma_gather":
                    nc.gpsimd.dma_gather(
                        out_ap=kv_g[:, 0:JB, :],
                        in_ap=kv_table[0:SPLIT_LO, :],
                        idxs_ap=ixdl_sb[:, 0:JBP // 16],
                        num_idxs=JBP, num_idxs_reg=JBP, elem_size=256)
                    nc.gpsimd.dma_gather(
                        out_ap=kv_g[:, JB:K, :],
                        in_ap=kv_table[SPLIT_HI:NTP, :],
                        idxs_ap=ixdl_sb[:, JBP // 16:IXC],
                        num_idxs=KP - JBP, num_idxs_reg=KP - JBP, elem_size=256)
                else:
                    idx32 = sb.tile([P, K], i32, tag="i32")
                    nc.sync.dma_start(out=idx32[:],
                                      in_=src_idx[u * P:(u + 1) * P, :])
                    for j in range(K):
                        nc.gpsimd.indirect_dma_start(
                            out=kv_g[:, j, :], out_offset=None,
                            in_=kv_table[:],
                            in_offset=bass.IndirectOffsetOnAxis(
                                ap=idx32[:, j:j + 1], axis=0))
                X = bigp.tile([P, K, 132], f16, tag="X")
                psS_t = psS.tile([P, 132], f32, tag="acc")
                for (g0, gsz) in groups:
                    pqe = psQ.tile([P, 8, P], f32, tag="pq")
                    for j in range(gsz):
                        c = g0 + j
                        nc.tensor.matmul(
                            pqe[:, j, :],
                            lhsT=oh[:, c * P:(c + 1) * P],
                            rhs=q_loc[:, u * D:(u + 1) * D],
                            start=True, stop=True)
                    qe = sb.tile([P, 8, P], f16, tag="qe")
                    nc.scalar.activation(out=qe[:, :gsz, :], in_=pqe[:, :gsz, :],
                                         func=COPY)
                    qk = sb.tile([P, 8, P], f16, tag="qk")
                    nc.vector.tensor_tensor(
                        out=qk[:, :gsz, :], in0=qe[:, :gsz, :],
                        in1=kv_g[:, g0:g0 + gsz, 0:128], op=MUL)
                    t16 = sb.tile([P, 8, 64], f16, tag="t16")
                    nc.vector.tensor_tensor(out=t16[:, :gsz, :],
                                            in0=qk[:, :gsz, 0:64],
                                            in1=qk[:, :gsz, 64:128], op=ADD)
                    t8 = sb.tile([P, 8, 32], f16, tag="t8")
                    nc.vector.tensor_tensor(out=t8[:, :gsz, :],
                                            in0=t16[:, :gsz, 0:32],
                                            in1=t16[:, :gsz, 32:64], op=ADD)
                    t4 = sb.tile([P, 8, 16], f16, tag="t4")
                    nc.vector.tensor_tensor(out=t4[:, :gsz, :],
                                            in0=t8[:, :gsz, 0:16],
                                            in1=t8[:, :gsz, 16:32], op=ADD)
                    t2 = sb.tile([P, 8, 8], f16, tag="t2")
                    nc.vector.tensor_tensor(out=t2[:, :gsz, :],
                                            in0=t4[:, :gsz, 0:8],
                                            in1=t4[:, :gsz, 8:16], op=ADD)
                    al = sb.tile([P, 8, 4], f16, tag="al")
                    nc.vector.tensor_tensor(out=al[:, :gsz, :],
                                            in0=t2[:, :gsz, 0:4],
                                            in1=t2[:, :gsz, 4:8], op=ADD)
                    nc.scalar.activation(out=X[:, g0:g0 + gsz, 128:132],
                                         in_=al[:, :gsz, :], func=EXP)
                    nc.vector.tensor_tensor(
                        out=X[:, g0:g0 + gsz, 0:128].rearrange(
                            "p a (e h) -> p a e h", h=H),
                        in0=kv_g[:, g0:g0 + gsz, 128:256].rearrange(
                            "p a (e h) -> p a e h", h=H),
                        in1=X[:, g0:g0 + gsz, 128:132, None].rearrange(
                            "p a h o -> p a o h").to_broadcast([P, gsz, DH, H]),
                        op=MUL)
                    for j in range(gsz):
                        c = g0 + j
                        nc.tensor.matmul(
                            psS_t[:, 0:132], lhsT=ohT[:, c, :], rhs=X[:, c, 0:132],
                            start=(c == 0), stop=(c == K - 1))
                dn = sb.tile([P, H], f32, tag="dn")
                nc.scalar.activation(out=dn[:], in_=psS_t[:, 128:132],
                                     func=COPY, bias=1e-16)
                rc = sb.tile([P, H], f32, tag="rc")
                nc.vector.reciprocal(out=rc[:], in_=dn[:])
                ot = sb.tile([P, D], f32, tag="ot")
                nc.vector.tensor_tensor(
                    out=ot[:].rearrange("p (e h) -> p e h", h=H),
                    in0=psS_t[:, 0:128].rearrange("p (e h) -> p e h", h=H),
                    in1=rc[:, :, None].rearrange("p h o -> p o h").to_broadcast(
                        [P, DH, H]),
                    op=MUL)
                nc.vector.tensor_tensor(
                    out=out_acc[:, u * D:(u + 1) * D], in0=ot[:],
                    in1=s_loc[:, u * D:(u + 1) * D], op=ADD)
            nc.sync.dma_start(out=out_t[:], in_=out_acc[:])

    nc.finalize()
    return nc


def _run(inputs, trace=False):
    _ensure_hooks()
    from concourse.bass_utils import run_bass_kernel_spmd

    meta = _prep(**inputs)
    key = (meta['TPC'], meta['NT'], meta['K'])
    if key not in _prog_cache:
        _prog_cache[key] = _build(*key)
    nc = _prog_cache[key]
    res = run_bass_kernel_spmd(nc, meta['in_maps'],
                               core_ids=list(range(NCORES)), trace=trace)
    TPC, D_ = meta['TPC'], D
    outs = []
    for c in range(NCORES):
        o = res.results[c]["out"]                      # [P, TPC*D]
        outs.append(o.reshape(P, TPC, D_).transpose(1, 0, 2).reshape(TPC * P, D_))
    out_perm = np.concatenate(outs, axis=0)            # [NT*P, D] (dh,h) cols
    unperm = np.empty_like(out_perm)
    unperm[:, meta['old']] = out_perm
    out = unperm[meta['node_slot']].astype(np.float32)
    return out, res


def kernel(**inputs) -> np.ndarray:
    out, _ = _run(inputs, trace=False)
    return out
